# revision 3
# baseline (speedup 1.0000x reference)
"""Trainium2 Bass kernel for nn_Conv2DRand: batchnorm (training-mode, batch
stats) + 3x3 SAME conv, NHWC, f32.

Full computation:
    mean/var over (N,H,W) per channel; x_bn = (x-mean)*rsqrt(var+eps) + beta
    out = conv2d(x_bn, kernels, SAME, stride 1, NHWC x HWIO -> NHWC)

Sharding: data-parallel over batch across 8 cores (8 images each); batch
statistics via a tiny cross-core AllReduce of [sum, sumsq] per channel.

Key trick: the BN affine transform is folded into the conv so the elementwise
BN pass over the full tensor disappears:
    out = conv(x_pad, K*s) + c
where s = rsqrt(var+eps), x is padded with padval = mean - beta/s (which makes
x_bn's zero-padding exact), and c[co] = sum_{tap,ci} (K*s)[ci,co] *
(beta/s - mean)[ci] restores the additive part uniformly.

Per core pipeline:
  Phase 1: stream x in [128px, 64ch] tiles; one matmul per tile with
           lhsT = x, rhs = [ones | x] accumulating [sums | x^T x] in PSUM.
           Diagonal of x^T x = per-channel sumsq. AllReduce [64,2] stats.
  Phase 2: per image, transpose rows to channel-major via TensorE into a
           padded [64, 114*114] buffer; 3x3 conv = 9 accumulating matmuls
           (lhsT = folded weights [ci,co], rhs = shifted windows) producing
           [co, 4 rows * 114]; bias-add on the PSUM->SBUF copy; transpose
           back per row via TensorE; DMA out.
"""

import numpy as np

import concourse.bass as bass
import concourse.tile as tile
from concourse import bacc, mybir
from concourse import bass_utils
from concourse.masks import make_identity

F32 = mybir.dt.float32

N_CORES = 8
N_FULL = 64          # full batch
H = 112
W = 112
C = 64
EPS = 1e-5
BW = W + 2           # padded row width: 114
RG = 4               # output rows per PSUM group (4*114 = 456 <= 512)
GROUPS = H // RG     # 28 groups per image
XT_LEN = 1 + BW * (H + 2) + 1   # lead pad + 114 padded rows + tail pad = 12998
P1_CHUNK = 49        # phase-1 pixel tiles per DMA chunk


def build_kernel(n_imgs: int, n_cores: int):
    """Build and compile the per-core Bass program."""
    npix = n_imgs * H * W
    tot = N_FULL * H * W  # global pixel count for the batch statistics

    nc = bacc.Bacc(
        "TRN2", target_bir_lowering=False, debug=False, num_devices=n_cores
    )
    x = nc.dram_tensor("x", [npix, C], F32, kind="ExternalInput").ap()
    kern = nc.dram_tensor("kern", [9, C, C], F32, kind="ExternalInput").ap()
    beta = nc.dram_tensor("beta", [C, 1], F32, kind="ExternalInput").ap()
    out = nc.dram_tensor("out", [npix, C], F32, kind="ExternalOutput").ap()

    with tile.TileContext(nc) as tc:
        _body(tc, out, x, kern, beta, n_imgs, n_cores, npix, tot)
    nc.compile()
    return nc


def _body(tc, out, x, kern, beta, n_imgs, n_cores, npix, tot):
    nc = tc.nc
    P = 128

    with (
        tc.tile_pool(name="singles", bufs=1) as singles,
        tc.tile_pool(name="small", bufs=1) as small,
        tc.tile_pool(name="p1", bufs=3) as p1pool,
        tc.tile_pool(name="xt", bufs=2) as xtpool,
        tc.tile_pool(name="slab", bufs=3) as slabpool,
        tc.tile_pool(name="ob", bufs=3) as obpool,
        tc.tile_pool(name="otb", bufs=3) as otbpool,
        tc.tile_pool(name="ps_stats", bufs=2, space="PSUM") as ps_stats,
        tc.tile_pool(name="ps_t", bufs=3, space="PSUM") as ps_t,
        tc.tile_pool(name="ps_o", bufs=2, space="PSUM") as ps_o,
        tc.tile_pool(name="ps_c", bufs=1, space="PSUM") as ps_c,
        tc.tile_pool(name="dram", bufs=2, space="DRAM") as dram,
    ):
        ident = singles.tile([P, P], F32)
        make_identity(nc, ident)

        # ---------------- Phase 1: local stats via TensorE ----------------
        # acc[:, 0] = sum_px x[px, ch]; acc[:, 1:65] = x^T x (diag = sumsq)
        acc = singles.tile([C, C + 1], F32)
        nc.vector.memset(acc, 0.0)

        a_tot = npix // P                       # pixel tiles of 128
        n_chunks = (a_tot + P1_CHUNK - 1) // P1_CHUNK
        xp = x.rearrange("(p a) c -> p a c", p=P)   # [128, a_tot, 64]
        for ci in range(n_chunks):
            a0 = ci * P1_CHUNK
            cw = min(P1_CHUNK, a_tot - a0)
            xt = p1pool.tile([P, P1_CHUNK, C + 1], F32, tag="p1")
            nc.vector.memset(xt[:, :cw, 0:1], 1.0)
            nc.sync.dma_start(out=xt[:, :cw, 1:], in_=xp[:, a0 : a0 + cw, :])
            ps = ps_stats.tile([C, C + 1], F32, tag="st")
            for j in range(cw):
                nc.tensor.matmul(
                    ps,
                    lhsT=xt[:, j, 1:],
                    rhs=xt[:, j, :],
                    start=(j == 0),
                    stop=(j == cw - 1),
                )
            nc.vector.tensor_add(acc, acc, ps)

        # sumsq = diag(x^T x) via identity mask + row reduce
        masked = small.tile([C, C], F32)
        nc.vector.tensor_mul(masked, acc[:, 1:], ident[:C, :C])
        loc = small.tile([C, 2], F32)
        nc.vector.tensor_copy(loc[:, 0:1], acc[:, 0:1])
        nc.vector.reduce_sum(loc[:, 1:2], masked, axis=mybir.AxisListType.X)

        # ---------------- AllReduce batch stats across cores ----------------
        cin = dram.tile([C, 2], F32)
        cout = dram.tile([C, 2], F32, addr_space="Shared")
        nc.sync.dma_start(out=cin, in_=loc)
        nc.gpsimd.collective_compute(
            "AllReduce",
            mybir.AluOpType.add,
            replica_groups=[list(range(n_cores))],
            ins=[cin[:].opt()],
            outs=[cout[:].opt()],
        )
        g = small.tile([C, 2], F32)
        nc.sync.dma_start(out=g, in_=cout)

        # ---------------- BN folding constants ----------------
        mean = small.tile([C, 1], F32)
        nc.vector.tensor_scalar_mul(mean, g[:, 0:1], 1.0 / tot)
        e2 = small.tile([C, 1], F32)
        nc.vector.tensor_scalar_mul(e2, g[:, 1:2], 1.0 / tot)
        msq = small.tile([C, 1], F32)
        nc.vector.tensor_mul(msq, mean, mean)
        var = small.tile([C, 1], F32)
        nc.vector.tensor_sub(var, e2, msq)
        eps_t = small.tile([C, 1], F32)
        nc.vector.memset(eps_t, EPS)
        std = small.tile([C, 1], F32)
        nc.scalar.activation(
            std, var, mybir.ActivationFunctionType.Sqrt, bias=eps_t, scale=1.0
        )
        s = small.tile([C, 1], F32)
        nc.vector.reciprocal(s, std)

        beta_sb = small.tile([C, 1], F32)
        nc.sync.dma_start(out=beta_sb, in_=beta)
        bstd = small.tile([C, 1], F32)
        nc.vector.tensor_mul(bstd, beta_sb, std)
        padval = small.tile([C, 1], F32)
        nc.vector.tensor_sub(padval, mean, bstd)   # pad x with mean - beta/s
        negpad = small.tile([C, 1], F32)
        nc.vector.tensor_sub(negpad, bstd, mean)

        # folded weights: Ws[ci, tap, co] = K[tap, ci, co] * s[ci]
        wt = singles.tile([C, 9, C], F32)
        nc.sync.dma_start(out=wt, in_=kern.rearrange("t i o -> i t o"))
        ws = singles.tile([C, 9, C], F32)
        nc.vector.tensor_scalar_mul(ws, wt, s)

        # output bias c[co] = sum_tap Ws[tap].T @ negpad
        cps = ps_c.tile([C, 1], F32)
        for t9 in range(9):
            nc.tensor.matmul(
                cps, lhsT=ws[:, t9, :], rhs=negpad, start=(t9 == 0), stop=(t9 == 8)
            )
        cbias = small.tile([C, 1], F32)
        nc.vector.tensor_copy(cbias, cps)

        # ---------------- Phase 2: conv per image ----------------
        x3 = x.rearrange("(r w) c -> r w c", w=W)    # [n_imgs*112, 112, 64]
        o3 = out.rearrange("(r w) c -> r w c", w=W)
        SLAB = 28                                    # rows per input DMA

        for img in range(n_imgs):
            xt_buf = xtpool.tile([C, XT_LEN], F32, tag="xt")
            # pads: top row (incl lead col), bottom row (incl tail col),
            # left/right columns of interior rows. value = padval[ci].
            top = xt_buf[:, 0 : 1 + BW]
            nc.vector.memset(top, 0.0)
            nc.vector.tensor_scalar_add(top, top, padval)
            bot = xt_buf[:, 1 + BW * (H + 1) : XT_LEN]
            nc.vector.memset(bot, 0.0)
            nc.vector.tensor_scalar_add(bot, bot, padval)
            interior = xt_buf[:, 1 + BW : 1 + BW * (H + 1)].rearrange(
                "p (r q) -> p r q", q=BW
            )
            for cc in (0, W + 1):
                side = interior[:, :, cc : cc + 1]
                nc.vector.memset(side, 0.0)
                nc.vector.tensor_scalar_add(side, side, padval)

            # rows -> channel-major via TensorE transpose
            for sl in range(H // SLAB):
                slab = slabpool.tile([W, SLAB, C], F32, tag="slab")
                r0 = img * H + sl * SLAB
                nc.sync.dma_start(
                    out=slab,
                    in_=x3[r0 : r0 + SLAB, :, :].rearrange("r w c -> w r c"),
                )
                for rr in range(SLAB):
                    r = sl * SLAB + rr
                    pst = ps_t.tile([C, W], F32, tag="t")
                    nc.tensor.transpose(pst, slab[:, rr, :], ident[:W, :W])
                    dst0 = 1 + BW * (r + 1) + 1
                    nc.scalar.activation(
                        xt_buf[:, dst0 : dst0 + W],
                        pst,
                        mybir.ActivationFunctionType.Copy,
                    )

            # 3x3 conv: 9 accumulating matmuls per 4-row output group
            for gi in range(GROUPS):
                po = ps_o.tile([C, RG * BW], F32, tag="o")
                for t9 in range(9):
                    dh, dw = divmod(t9, 3)
                    off = gi * RG * BW + dh * BW + dw
                    nc.tensor.matmul(
                        po,
                        lhsT=ws[:, t9, :],
                        rhs=xt_buf[:, off : off + RG * BW],
                        start=(t9 == 0),
                        stop=(t9 == 8),
                    )
                ob = obpool.tile([C, RG, W], F32, tag="ob")
                pov = po.rearrange("p (r q) -> p r q", q=BW)
                nc.scalar.activation(
                    ob,
                    pov[:, :, 1 : 1 + W],
                    mybir.ActivationFunctionType.Identity,
                    bias=cbias,
                )
                otb = otbpool.tile([W, RG, C], F32, tag="otb")
                for rr in range(RG):
                    pot = ps_t.tile([W, C], F32, tag="t")
                    nc.tensor.transpose(pot, ob[:, rr, :], ident[:C, :C])
                    nc.vector.tensor_copy(otb[:, rr, :], pot)
                ro = img * H + gi * RG
                nc.sync.dma_start(
                    out=o3[ro : ro + RG, :, :].rearrange("r w c -> w r c"),
                    in_=otb,
                )


_CACHE = {}


def _get_kernel(n_imgs, n_cores):
    key = (n_imgs, n_cores)
    if key not in _CACHE:
        _CACHE[key] = build_kernel(n_imgs, n_cores)
    return _CACHE[key]


def kernel(x, kernels, beta):
    """Full inputs -> full output. Shards batch over 8 NeuronCores."""
    n = x.shape[0]
    per = n // N_CORES
    npix = per * H * W
    nc = _get_kernel(per, N_CORES)

    kern9 = np.ascontiguousarray(kernels.reshape(9, C, C), dtype=np.float32)
    beta2 = np.ascontiguousarray(beta.reshape(C, 1), dtype=np.float32)
    in_maps = []
    for ci in range(N_CORES):
        xs = np.ascontiguousarray(
            x[ci * per : (ci + 1) * per].reshape(npix, C), dtype=np.float32
        )
        in_maps.append({"x": xs, "kern": kern9, "beta": beta2})

    res = bass_utils.run_bass_kernel_spmd(
        nc, in_maps, core_ids=list(range(N_CORES)), trace=TRACE
    )
    global LAST_RESULTS
    LAST_RESULTS = res
    outs = [
        res.results[ci]["out"].reshape(per, H, W, C) for ci in range(N_CORES)
    ]
    return np.concatenate(outs, axis=0)


TRACE = False
LAST_RESULTS = None
